# revision 30
# baseline (speedup 1.0000x reference)
"""DCE-modulated ResBlock (dense_cnn) on 8 Trainium2 NeuronCores.

Data-parallel over batch (16 images -> 2 per core), weights replicated.
BatchNorm batch statistics are exact via cross-core AllReduces (sync-BN).

v2 layout vs the f32r baseline:
  - all conv operands (x, w1, w2, wsc, t1) in bf16: halves input DMA and
    SBUF footprint; PE rate is 1 cycle/row for both f32r and bf16.
  - per-image gating: conv1 on image 0 starts as soon as image 0 has
    landed + its modulation gate is done, instead of waiting for all
    input DMA.
  - shortcut 1x1 conv moved between conv1 and conv2 so its PE work hides
    the bn1 AllReduce latency.
  - conv2 and shortcut VALUES are stored (bf16) during the stats passes,
    so the final pass has no matmuls: it is pure DVE/ACT/DMA and is
    paced by the y writeout.
"""

from contextlib import ExitStack

import numpy as np

import concourse.bass as bass
import concourse.mybir as mybir
from concourse import tile
from concourse.bass_utils import run_bass_kernel_spmd

F32 = mybir.dt.float32
F32R = mybir.dt.float32r
F16 = mybir.dt.float16
AF = mybir.ActivationFunctionType
ALU = mybir.AluOpType

B, C, H, W = 16, 256, 64, 64
LDCE, CDCE = 100, 128
NCORES = 8
NB = B // NCORES          # images per core
MT = C // 128             # channel tiles (2)
PW = W + 2                # padded row width 66
PLEN = (H + 2) * PW + 2   # padded buffer + 2 guard cols (4358)
RG = 8                    # row groups per image
RGR = H // RG             # rows per group (8)
TLEN = RGR * W            # columns per psum tile (512)
TSP = RGR * PW            # padded columns spanned per group (528)
NLOC = NB * H * W         # local reduction count per channel (8192)
NGLB = B * H * W          # global reduction count (65536)
EPS = 1e-5


def _split_sync_waits(nc, max_waits=1):
    """This container's walrus build accepts only one sync-wait command per
    instruction; hoist excess waits onto same-engine NoOps placed before."""
    for f in nc.m.functions:
        for bb in f.blocks:
            insts = bb.instructions
            if not any(
                i.sync_info is not None and len(i.sync_info.on_wait) > max_waits
                for i in insts
            ):
                continue
            newlist = []
            for inst in insts:
                si = inst.sync_info
                if si is not None and len(si.on_wait) > max_waits:
                    waits = list(si.on_wait)
                    extra, keep = waits[:-max_waits], waits[-max_waits:]
                    for j in range(0, len(extra), max_waits):
                        nop = mybir.InstNoOp(name=f"{inst.name}-sw{j}", ins=[], outs=[])
                        nop.engine = inst.engine
                        nop.sync_info = mybir.SyncInfo(
                            on_wait=extra[j : j + max_waits], on_update=[]
                        )
                        newlist.append(nop)
                    inst.sync_info = mybir.SyncInfo(
                        on_wait=keep, on_update=list(si.on_update)
                    )
                newlist.append(inst)
            bb.instructions = newlist


def _bn_stats_raw(nc, out_ap, in_ap):
    """One HW BNStats chunk (count/mean/count*var for even+odd lanes) over
    the full (possibly strided) input AP; bass's shape assert only allows
    flat inputs, walrus only allows 6 outputs, so emit the IR directly."""
    eng = nc.vector
    eng.add_instruction(
        mybir.InstBNStats(
            name=nc.get_next_instruction_name(),
            ins=[eng.lower_ap(in_ap)],
            outs=[eng.lower_ap(out_ap)],
        )
    )


def _build():
    nc = bass.Bass(
        "TRN2",
        target_bir_lowering=False,
        debug=False,
        num_devices=NCORES,
        use_seq_codegen=True,
        num_swdge_queues=4,
    )

    # ---- kernel I/O (per-core shapes) ----
    xp_d = nc.dram_tensor("xp", [NB, C, PLEN], F16, kind="ExternalInput")
    dce_d = nc.dram_tensor("dce", [NB, LDCE, CDCE], F32, kind="ExternalInput")
    w1t_d = nc.dram_tensor("w1t", [MT, MT, 128, 9 * 128], F16, kind="ExternalInput")
    w2t_d = nc.dram_tensor("w2t", [C, C], F16, kind="ExternalInput")
    wsct_d = nc.dram_tensor("wsct", [C, C], F16, kind="ExternalInput")
    wdce_d = nc.dram_tensor("wdce_t", [CDCE, C], F32, kind="ExternalInput")
    wst_d = nc.dram_tensor("wst", [C, C // 2], F32, kind="ExternalInput")
    wet_d = nc.dram_tensor("wet", [C // 2, C], F32, kind="ExternalInput")
    chc_d = nc.dram_tensor("chc", [C, 9], F32, kind="ExternalInput")
    # per-channel vectors: [b_dce, g1, be1, g2, be2, gs, bes, b_expand]
    chv_d = nc.dram_tensor("chv", [C, 8], F32, kind="ExternalInput")
    bsh_d = nc.dram_tensor("bsh", [C // 2], F32, kind="ExternalInput")
    eye_d = nc.dram_tensor("eye", [128, 128], F16, kind="ExternalInput")
    y_d = nc.dram_tensor("y", [NB, C, H, W], F32, kind="ExternalOutput")

    # collective bounce buffers (one pair per AllReduce so they pipeline)
    cc1_in = {mt: nc.dram_tensor(f"cc1_in{mt}", [128, 2], F32) for mt in range(MT)}
    cc1_out = {
        mt: nc.dram_tensor(f"cc1_out{mt}", [128, 2], F32, addr_space="Shared")
        for mt in range(MT)
    }
    cc2_in = {mt: nc.dram_tensor(f"cc2_in{mt}", [128, 4], F32) for mt in range(MT)}
    cc2_out = {
        mt: nc.dram_tensor(f"cc2_out{mt}", [128, 4], F32, addr_space="Shared")
        for mt in range(MT)
    }
    groups = [list(range(NCORES))]

    with tile.TileContext(nc) as tc, ExitStack() as es:
        pers = es.enter_context(tc.tile_pool(name="pers", bufs=1))
        stage = es.enter_context(tc.tile_pool(name="stage", bufs=4))

        # ---- persistent SBUF buffers ----
        xm = {}   # padded x, later x*mod (fp16)
        t1 = {}   # conv1 out raw, later silu(bn1(.)) (fp16)
        scb = {}  # shortcut conv raw values (fp16)
        yst = {}  # f32 staging for y so DMAs go out in big contiguous runs
        for b in range(NB):
            yst[b] = pers.tile([128, H * W], F32, tag=f"yst{b}", name=f"yst{b}")
            for ct in range(MT):
                xm[b, ct] = pers.tile([128, PLEN], F16, tag=f"xm{b}{ct}", name=f"xm{b}{ct}")
                t1[b, ct] = pers.tile([128, H * W], F16, tag=f"t1{b}{ct}", name=f"t1{b}{ct}")
                scb[b, ct] = pers.tile([128, H * W], F16, tag=f"sc{b}{ct}", name=f"sc{b}{ct}")

        # ---- warm the ACT spline tables (sigmoid/silu) before they hit the
        # gate's critical path; runs under the input DMA.
        warm = pers.tile([128, 1], F32, tag="warm", name="warm")
        nc.vector.memset(warm[:], 0.0)
        nc.scalar.activation(warm[:], warm[:], AF.Sigmoid)
        nc.scalar.activation(warm[:], warm[:], AF.Silu)
        nc.scalar.activation(warm[:], warm[:], AF.Relu)

        # ---- dce first (gates the earliest PE work), then image 0 of x in
        # 6 chunks across all six DMA streams (4 swdge + sync + scalar),
        # then gate smalls, conv1 weights, image 1.
        dce_sb = {}
        for b in range(NB):
            dce_sb[b] = pers.tile([LDCE, CDCE], F32, tag=f"dce{b}", name=f"dce{b}")
            nc.sync.dma_start(dce_sb[b][:], dce_d[b, :, :])
        XT = [0, 2180, PLEN]
        for i in range(2):
            s, e = XT[i], XT[i + 1]
            nc.gpsimd.dma_start(xm[0, 0][:, s:e], xp_d[0, 0:128, s:e])
            nc.gpsimd.dma_start(xm[0, 1][:, s:e], xp_d[0, 128:256, s:e])
        wdce = {}
        wet = {}
        chv = {}
        chc = {}
        for mt in range(MT):
            wdce[mt] = pers.tile([128, 128], F32, tag=f"wdce{mt}", name=f"wdce{mt}")
            nc.sync.dma_start(wdce[mt][:], wdce_d[:, mt * 128 : mt * 128 + 128])
            wet[mt] = pers.tile([128, 128], F32, tag=f"wet{mt}", name=f"wet{mt}")
            nc.sync.dma_start(wet[mt][:], wet_d[:, mt * 128 : mt * 128 + 128])
            chv[mt] = pers.tile([128, 8], F32, tag=f"chv{mt}", name=f"chv{mt}")
            nc.sync.dma_start(chv[mt][:], chv_d[mt * 128 : mt * 128 + 128, :])
            chc[mt] = pers.tile([128, 9], F32, tag=f"chc{mt}", name=f"chc{mt}")
            nc.sync.dma_start(chc[mt][:], chc_d[mt * 128 : mt * 128 + 128, :])
        wst = {}
        for kt in range(MT):
            wst[kt] = pers.tile([128, 128], F32, tag=f"wst{kt}", name=f"wst{kt}")
            nc.sync.dma_start(wst[kt][:], wst_d[kt * 128 : kt * 128 + 128, :])
        bsh = pers.tile([128, 1], F32, tag="bsh", name="bsh")
        nc.sync.dma_start(bsh[:], bsh_d[:].rearrange("(p a) -> p a", a=1))
        eye_sb = pers.tile([128, 128], F16, tag="eye", name="eye")
        nc.sync.dma_start(eye_sb[:], eye_d[:, :])

        w1 = {}
        for mt in range(MT):      # mt outer: conv1(mt0) weights land first
            for kt in range(MT):
                big = pers.tile([128, 9 * 128], F16, tag=f"w1b{kt}{mt}", name=f"w1b{kt}{mt}")
                nc.gpsimd.dma_start(big[:], w1t_d[kt, mt, :, :])
                for tap in range(9):
                    w1[tap, kt, mt] = big[:, tap * 128 : (tap + 1) * 128]

        w2 = {}
        wsc = {}
        for kt in range(MT):
            bw = pers.tile([128, 2 * 128], F16, tag=f"w2b{kt}", name=f"w2b{kt}")
            nc.gpsimd.dma_start(
                bw[:].rearrange("p (m o) -> p m o", m=MT),
                w2t_d[kt * 128 : kt * 128 + 128, :].rearrange("c (m o) -> c m o", m=MT),
            )
            bs = pers.tile([128, 2 * 128], F16, tag=f"wscb{kt}", name=f"wscb{kt}")
            nc.gpsimd.dma_start(
                bs[:].rearrange("p (m o) -> p m o", m=MT),
                wsct_d[kt * 128 : kt * 128 + 128, :].rearrange("c (m o) -> c m o", m=MT),
            )
            for mt in range(MT):
                w2[kt, mt] = bw[:, mt * 128 : (mt + 1) * 128]
                wsc[kt, mt] = bs[:, mt * 128 : (mt + 1) * 128]
        XH = [0, 2180, PLEN]
        for i in range(2):
            s, e = XH[i], XH[i + 1]
            nc.sync.dma_start(xm[1, 0][:, s:e], xp_d[1, 0:128, s:e])
            nc.scalar.dma_start(xm[1, 1][:, s:e], xp_d[1, 128:256, s:e])


        esB = ExitStack()
        psB = esB.enter_context(tc.tile_pool(name="psB", bufs=2, space="PSUM"))

        # =====================================================================
        # Phase A: modulation gate, per image (tiny matmuls borrow the
        # "sc"-tag psum tiles; the sc pass only starts much later)
        # =====================================================================
        def tiny_ps():
            return psB.tile([128, TLEN], F32, tag="sc", bufs=3, name="tiny")

        # ones vector for sequence-mean matmul (f32r via ACT rounding)
        ones_f = pers.tile([128, 1], F32, tag="ones_f", name="ones_f")
        nc.vector.memset(ones_f[:], 1.0)

        pooled = pers.tile([128, NB], F32, tag="pooled", name="pooled")
        sp = {}
        m_r = {}
        mod = {}
        for ct in range(MT):
            sp[ct] = pers.tile([128, NB], F32, tag=f"sp{ct}", name=f"sp{ct}")
            m_r[ct] = pers.tile([128, NB], F32, tag=f"m{ct}", name=f"m{ct}")
            mod[ct] = pers.tile([128, NB], F32, tag=f"mod{ct}", name=f"mod{ct}")
        h_r = pers.tile([128, NB], F32, tag="h_r", name="h_r")

        def gate_dve(b):
            # spatial_proj via border-sum identity; gath cols:
            # [S, rowE, row0, colE, col0, x(E,E), x(E,0), x(0,E), x(0,0)]
            for ct in range(MT):
                buf = xm[b, ct]
                gath = stage.tile([128, 9], F32, tag="gath", name="gath")
                rows = stage.tile([128, H], F32, tag="rows", name="rows")
                halfA = buf[:, 67 : 67 + 32 * PW].rearrange(
                    "p (r c) -> p r c", r=32
                )[:, :, 0:W]
                halfB = buf[:, 67 + 32 * PW : 67 + 64 * PW].rearrange(
                    "p (r c) -> p r c", r=32
                )[:, :, 0:W]
                nc.vector.reduce_sum(rows[:, 0:32], halfA, axis=mybir.AxisListType.X)
                nc.vector.reduce_sum(rows[:, 32:64], halfB, axis=mybir.AxisListType.X)
                nc.vector.reduce_sum(gath[:, 0:1], rows[:], axis=mybir.AxisListType.X)
                nc.vector.tensor_copy(gath[:, 1:2], rows[:, H - 1 : H])
                nc.vector.tensor_copy(gath[:, 2:3], rows[:, 0:1])
                colE = buf[:, 67 + W - 1 : 67 + W - 1 + H * PW].rearrange(
                    "p (r c) -> p r c", r=H
                )[:, :, 0:1]
                col0 = buf[:, 67 : 67 + H * PW].rearrange(
                    "p (r c) -> p r c", r=H
                )[:, :, 0:1]
                nc.vector.reduce_sum(gath[:, 3:4], colE, axis=mybir.AxisListType.XY)
                nc.vector.reduce_sum(gath[:, 4:5], col0, axis=mybir.AxisListType.XY)
                be = 67 + (H - 1) * PW
                nc.vector.tensor_copy(gath[:, 5:6], buf[:, be + W - 1 : be + W])
                nc.vector.tensor_copy(gath[:, 6:7], buf[:, be : be + 1])
                nc.vector.tensor_copy(gath[:, 7:8], buf[:, 67 + W - 1 : 67 + W])
                nc.vector.tensor_copy(gath[:, 8:9], buf[:, 67 : 68])
                gm = stage.tile([128, 9], F32, tag="gm", name="gm")
                nc.vector.tensor_tensor(gm[:], gath[:], chc[ct][:], op=ALU.mult)
                nc.vector.reduce_sum(
                    sp[ct][:, b : b + 1], gm[:], axis=mybir.AxisListType.X
                )

        # chunk boundaries chosen so conv1's early windows unblock first
        XCH = [0, 1190, 2180, 3270, PLEN]

        def gate_pe(b):
            # dce sequence mean via matmul with ones
            ps = tiny_ps()
            nc.tensor.matmul(ps[:, 0:1], dce_sb[b][:], ones_f[0:LDCE, :], start=True, stop=True)
            nc.scalar.mul(pooled[:, b : b + 1], ps[:, 0:1], 1.0 / LDCE)
            # m = (dce_pooled @ w_dce.T + b_dce) * spatial_proj
            for mt in range(MT):
                ps2 = tiny_ps()
                nc.tensor.matmul(
                    ps2[:, 0:1], wdce[mt][:], pooled[:, b : b + 1], start=True, stop=True
                )
                dcep = stage.tile([128, 1], F32, tag="dcep", name="dcep")
                nc.scalar.add(dcep[:], ps2[:, 0:1], chv[mt][:, 0:1])
                nc.vector.tensor_tensor(
                    m_r[mt][:, b : b + 1], dcep[:], sp[mt][:, b : b + 1], op=ALU.mult
                )
            # h = relu(m @ w_shrink.T + b_shrink)
            ps_h = tiny_ps()
            for kt in range(MT):
                nc.tensor.matmul(
                    ps_h[:, 0:1], wst[kt][:], m_r[kt][:, b : b + 1],
                    start=(kt == 0), stop=(kt == MT - 1),
                )
            nc.scalar.activation(h_r[:, b : b + 1], ps_h[:, 0:1], AF.Relu, bias=bsh[:])
            # mod = sigmoid(h @ w_expand.T + b_expand)
            for mt in range(MT):
                ps3 = tiny_ps()
                nc.tensor.matmul(
                    ps3[:, 0:1], wet[mt][:], h_r[:, b : b + 1], start=True, stop=True
                )
                nc.scalar.activation(
                    mod[mt][:, b : b + 1], ps3[:, 0:1], AF.Sigmoid, bias=chv[mt][:, 7:8]
                )
            # xm = x * mod (in place, chunked)
            for ct in range(MT):
                for i in range(len(XCH) - 1):
                    s, e = XCH[i], XCH[i + 1]
                    nc.vector.tensor_scalar_mul(
                        xm[b, ct][:, s:e], xm[b, ct][:, s:e], mod[ct][:, b : b + 1]
                    )

        # =====================================================================
        # Phase B: conv1 (+bn1 stats) -> sc pass -> conv2 pass
        # =====================================================================
        bnb1 = {mt: pers.tile([128, NB * RG, 6], F32, tag=f"bnb1{mt}", name=f"bnb1{mt}") for mt in range(MT)}
        bnbs = {mt: pers.tile([128, NB * RG, 6], F32, tag=f"bnbs{mt}", name=f"bnbs{mt}") for mt in range(MT)}
        bnb2 = {mt: pers.tile([128, NB * RG, 6], F32, tag=f"bnb2{mt}", name=f"bnb2{mt}") for mt in range(MT)}

        taps = [((kh - 1) * PW + (kw - 1), 3 * kh + kw) for kh in range(3) for kw in range(3)]

        def win(buf, rg, off=0):
            s = 67 + rg * TSP + off
            return buf[:, s : s + RGR * PW].rearrange("p (r c) -> p r c", r=RGR)[
                :, :, 0:W
            ]

        def conv1_mt_b(mt, b):
            # copies: ACT for mt0 (DVE light), DVE for mt1 (ACT does silu(0))
            for rg in range(RG):
                ps = psB.tile([128, TLEN], F32, tag="c1", name="c1", bufs=2)
                first = True
                for kt in range(MT):
                    for off, tap in taps:
                        nc.tensor.matmul(
                            ps[:],
                            w1[tap, kt, mt],
                            win(xm[b, kt], rg, off),
                            start=first,
                            stop=(kt == MT - 1 and tap == 8),
                        )
                        first = False
                _bn_stats_raw(nc, bnb1[mt][:, b * RG + rg, :], ps[:])
                dst = t1[b, mt][:, rg * TLEN : (rg + 1) * TLEN]
                if mt == 0:
                    nc.scalar.copy(dst, ps[:])
                else:
                    nc.vector.tensor_copy(dst, ps[:])

        # local chunk stats -> (sum, sum_x2) packed for the allreduce
        def local_sums(bnb, mt, dst_sum, dst_ex2):
            mv = stage.tile([128, 2], F32, tag="mv", name="mv")
            nc.vector.bn_aggr(
                mv[:],
                bnb[mt][:]
                .rearrange("p a s -> p (a s)")
                .rearrange("p (a b) -> p a b", b=3),
            )
            nc.vector.tensor_scalar_mul(dst_sum, mv[:, 0:1], float(NLOC))
            t = stage.tile([128, 1], F32, tag="tloc", name="tloc")
            nc.vector.tensor_tensor(t[:], mv[:, 0:1], mv[:, 0:1], op=ALU.mult)
            nc.vector.tensor_tensor(t[:], t[:], mv[:, 1:2], op=ALU.add)
            nc.vector.tensor_scalar_mul(dst_ex2, t[:], float(NLOC))

        # global bn affine: a = g*rsqrt(var+eps), c = be - mean*a
        def bn_affine(sum_ap, ex2_ap, g_ap, be_ap, a_dst, c_dst):
            mean = stage.tile([128, 1], F32, tag="bnm", name="bnm")
            nc.scalar.mul(mean[:], sum_ap, 1.0 / NGLB)
            var = stage.tile([128, 1], F32, tag="bnv", name="bnv")
            nc.scalar.mul(var[:], ex2_ap, 1.0 / NGLB)
            t = stage.tile([128, 1], F32, tag="bnt", name="bnt")
            nc.vector.tensor_tensor(t[:], mean[:], mean[:], op=ALU.mult)
            nc.vector.tensor_tensor(var[:], var[:], t[:], op=ALU.subtract)
            nc.vector.tensor_scalar_add(var[:], var[:], EPS)
            nc.vector.reciprocal(var[:], var[:])
            nc.scalar.sqrt(var[:], var[:])
            nc.vector.tensor_tensor(a_dst, var[:], g_ap, op=ALU.mult)
            nc.vector.tensor_tensor(t[:], mean[:], a_dst, op=ALU.mult)
            nc.vector.tensor_tensor(c_dst, be_ap, t[:], op=ALU.subtract)

        ar1 = {mt: pers.tile([128, 2], F32, tag=f"ar1{mt}", name=f"ar1{mt}") for mt in range(MT)}
        g1s = {mt: pers.tile([128, 2], F32, tag=f"g1s{mt}", name=f"g1s{mt}") for mt in range(MT)}
        a1 = {mt: pers.tile([128, 1], F32, tag=f"a1{mt}", name=f"a1{mt}") for mt in range(MT)}
        c1 = {mt: pers.tile([128, 1], F32, tag=f"c1v{mt}", name=f"c1v{mt}") for mt in range(MT)}

        def ar1_pre(mt):
            local_sums(bnb1, mt, ar1[mt][:, 0:1], ar1[mt][:, 1:2])
            nc.gpsimd.dma_start(cc1_in[mt][:], ar1[mt][:])
            nc.gpsimd.collective_compute(
                "AllReduce", ALU.add, replica_groups=groups,
                ins=[cc1_in[mt][:]], outs=[cc1_out[mt][:]],
            )

        def ar1_post(mt):
            nc.gpsimd.dma_start(g1s[mt][:], cc1_out[mt][:])
            bn_affine(
                g1s[mt][:, 0:1], g1s[mt][:, 1:2],
                chv[mt][:, 1:2], chv[mt][:, 2:3], a1[mt][:], c1[mt][:],
            )

        def silu_mt0():
            # ACT-only, big chunks; runs under conv1(mt1) whose copies are DVE
            for b in range(NB):
                for i in range(2):
                    s = t1[b, 0][:, i * 2048 : (i + 1) * 2048]
                    nc.scalar.activation(s, s, AF.Silu, bias=c1[0][:], scale=a1[0][:])

        def sc_pass():
            # 1x1 shortcut conv: values stored, stats kept; AR1-independent
            for b in range(NB):
                for rg in range(RG):
                    for mt in range(MT):
                        ps = psB.tile([128, TLEN], F32, tag="sc", bufs=3, name="sc")
                        for kt in range(MT):
                            nc.tensor.matmul(
                                ps[:], wsc[kt, mt], win(xm[b, kt], rg),
                                start=(kt == 0), stop=(kt == MT - 1),
                            )
                        _bn_stats_raw(nc, bnbs[mt][:, b * RG + rg, :], ps[:])
                        nc.scalar.copy(
                            scb[b, mt][:, rg * TLEN : (rg + 1) * TLEN], ps[:]
                        )

        def conv2_mt(mt):
            # stats-only pass: z2 values are recomputed in the final pass
            for b in range(NB):
                for rg in range(RG):
                    if mt == 0:
                        # silu for the second half of t1, chunk-interleaved
                        s = t1[b, 1][:, rg * TLEN : (rg + 1) * TLEN]
                        nc.scalar.activation(s, s, AF.Silu, bias=c1[1][:], scale=a1[1][:])
                    ps = psB.tile([128, TLEN], F32, tag="z2", name="z2", bufs=3)
                    for kt in range(MT):
                        nc.tensor.matmul(
                            ps[:], w2[kt, mt],
                            t1[b, kt][:, rg * TLEN : (rg + 1) * TLEN],
                            start=(kt == 0), stop=(kt == MT - 1),
                        )
                    _bn_stats_raw(nc, bnb2[mt][:, b * RG + rg, :], ps[:])

        ar2 = {mt: pers.tile([128, 4], F32, tag=f"ar2{mt}", name=f"ar2{mt}") for mt in range(MT)}
        g2s = {mt: pers.tile([128, 4], F32, tag=f"g2s{mt}", name=f"g2s{mt}") for mt in range(MT)}
        a2 = {mt: pers.tile([128, 1], F32, tag=f"a2{mt}", name=f"a2{mt}") for mt in range(MT)}
        asc = {mt: pers.tile([128, 1], F32, tag=f"as{mt}", name=f"as{mt}") for mt in range(MT)}
        ccb = {mt: pers.tile([128, 1], F32, tag=f"ccb{mt}", name=f"ccb{mt}") for mt in range(MT)}

        def ar2_pre(mt):
            local_sums(bnb2, mt, ar2[mt][:, 0:1], ar2[mt][:, 1:2])
            local_sums(bnbs, mt, ar2[mt][:, 2:3], ar2[mt][:, 3:4])
            nc.gpsimd.dma_start(cc2_in[mt][:], ar2[mt][:])
            nc.gpsimd.collective_compute(
                "AllReduce", ALU.add, replica_groups=groups,
                ins=[cc2_in[mt][:]], outs=[cc2_out[mt][:]],
            )

        drat = {mt: pers.tile([128, 128], F16, tag=f"dr{mt}", name=f"dr{mt}") for mt in range(MT)}

        def ar2_post(mt):
            # batched affine for bn2 (cols 0,1) and bns (cols 2,3):
            # a = g*rsqrt(var+eps), c = be - mean*a, done on [128,2] slices
            nc.gpsimd.dma_start(g2s[mt][:], cc2_out[mt][:])
            g = g2s[mt][:]
            mn = stage.tile([128, 2], F32, tag="bnm2", name="bnm2")
            nc.scalar.mul(mn[:], g.rearrange("p (s k) -> p s k", k=2)[:, :, 0], 1.0 / NGLB)
            vr = stage.tile([128, 2], F32, tag="bnv2", name="bnv2")
            nc.scalar.mul(vr[:], g.rearrange("p (s k) -> p s k", k=2)[:, :, 1], 1.0 / NGLB)
            t = stage.tile([128, 2], F32, tag="bnt2", name="bnt2")
            nc.vector.tensor_tensor(t[:], mn[:], mn[:], op=ALU.mult)
            nc.vector.tensor_tensor(vr[:], vr[:], t[:], op=ALU.subtract)
            nc.vector.tensor_scalar_add(vr[:], vr[:], EPS)
            av = stage.tile([128, 2], F32, tag="bna2", name="bna2")
            nc.vector.reciprocal(av[:], vr[:])
            nc.scalar.sqrt(av[:], av[:])
            gg = stage.tile([128, 2], F32, tag="bng2", name="bng2")
            nc.vector.tensor_copy(gg[:, 0:1], chv[mt][:, 3:4])
            nc.vector.tensor_copy(gg[:, 1:2], chv[mt][:, 5:6])
            nc.vector.tensor_tensor(av[:], av[:], gg[:], op=ALU.mult)
            nc.vector.tensor_copy(a2[mt][:], av[:, 0:1])
            nc.vector.tensor_copy(asc[mt][:], av[:, 1:2])
            nc.vector.tensor_tensor(t[:], mn[:], av[:], op=ALU.mult)  # mean*a
            nc.vector.tensor_copy(gg[:, 0:1], chv[mt][:, 4:5])
            nc.vector.tensor_copy(gg[:, 1:2], chv[mt][:, 6:7])
            nc.vector.tensor_tensor(t[:], gg[:], t[:], op=ALU.subtract)  # c2|cs
            nc.vector.reduce_sum(ccb[mt][:], t[:], axis=mybir.AxisListType.X)
            rt = stage.tile([128, 1], F32, tag="rt", name="rt")
            nc.vector.reciprocal(rt[:], av[:, 0:1])
            nc.vector.tensor_tensor(rt[:], rt[:], av[:, 1:2], op=ALU.mult)
            nc.vector.tensor_scalar_mul(drat[mt][:], eye_sb[:], rt[:])

        # =====================================================================
        # Phase C: final pass  y = silu(a2*(W2@t1s + diag(asc/a2)@sc) + ccb)
        # PE matmuls into PSUM, ACT silu (scale=a2, bias=ccb), DMA out
        # =====================================================================
        def final_mt(mt):
            for b in range(NB):
                for rg in range(RG):
                    sl = slice(rg * TLEN, (rg + 1) * TLEN)
                    ps = psB.tile([128, TLEN], F32, tag="z2", name="fz", bufs=3)
                    for kt in range(MT):
                        nc.tensor.matmul(
                            ps[:], w2[kt, mt], t1[b, kt][:, sl],
                            start=(kt == 0), stop=False,
                        )
                    nc.tensor.matmul(ps[:], drat[mt][:], scb[b, mt][:, sl], start=False, stop=True)
                    nc.scalar.activation(
                        yst[b][:, sl], ps[:], AF.Silu, bias=ccb[mt][:], scale=a2[mt][:]
                    )
                    if rg in (3, RG - 1):
                        h0 = 0 if rg == 3 else H // 2
                        eng = nc.gpsimd
                        eng.dma_start(
                            y_d[b, mt * 128 : mt * 128 + 128, h0 : h0 + H // 2, :],
                            yst[b][:, h0 * W : (h0 + H // 2) * W],
                        )

        # ---------------- emission schedule ----------------
        gate_dve(0)
        gate_pe(0)
        conv1_mt_b(0, 0)
        gate_dve(1)          # after b0 stats in the DVE FIFO (xm[1] landed)
        gate_pe(1)           # small PE bubble while b1's gate chain resolves
        conv1_mt_b(0, 1)
        ar1_pre(0)           # AllReduce(bn1 mt0) flies under conv1(mt1)
        conv1_mt_b(1, 0)
        conv1_mt_b(1, 1)
        ar1_post(0)
        silu_mt0()           # ACT; overlaps conv1(mt1) tail + sc pass
        ar1_pre(1)
        sc_pass()            # PE work hiding AllReduce(bn1 mt1)
        ar1_post(1)
        conv2_mt(0)          # silu(mt1) chunks interleaved inside
        ar2_pre(0)
        conv2_mt(1)
        ar2_post(0)
        final_mt(0)          # overlaps conv2(mt1) + AllReduce#2(mt1)
        ar2_pre(1)
        ar2_post(1)
        final_mt(1)
        esB.close()

    _split_sync_waits(nc)
    return nc


_NC = None


def _prep_inputs(inputs):
    f16 = np.float16
    w_conv1 = inputs["w_conv1"]  # (C, C, 3, 3) OIHW
    # [kt, mt, cin128, tap(kh*3+kw)*cout128] so each (kt, mt) block is one
    # contiguous DMA
    w1t = np.ascontiguousarray(
        w_conv1.transpose(2, 3, 1, 0)       # (kh, kw, cin, cout)
        .reshape(9, MT, 128, MT, 128)       # (tap, kt, cin, mt, cout)
        .transpose(1, 3, 2, 0, 4)           # (kt, mt, cin, tap, cout)
        .reshape(MT, MT, 128, 9 * 128)
    ).astype(f16)
    w2t = np.ascontiguousarray(inputs["w_conv2"][:, :, 0, 0].T).astype(f16)
    wsct = np.ascontiguousarray(inputs["w_sc"][:, :, 0, 0].T).astype(f16)
    wdce_t = np.ascontiguousarray(inputs["w_dce"].T).astype(np.float32)
    wst = np.ascontiguousarray(inputs["w_shrink"].T).astype(np.float32)
    wet = np.ascontiguousarray(inputs["w_expand"].T).astype(np.float32)

    wch = inputs["w_ch"][:, 0]  # (C, 3, 3)
    # gath order: [S, rowE, row0, colE, col0, x(E,E), x(E,0), x(0,E), x(0,0)]
    chc = np.stack(
        [
            wch.sum((1, 2)),
            -wch[:, 0, :].sum(1),
            -wch[:, 2, :].sum(1),
            -wch[:, :, 0].sum(1),
            -wch[:, :, 2].sum(1),
            wch[:, 0, 0],
            wch[:, 0, 2],
            wch[:, 2, 0],
            wch[:, 2, 2],
        ],
        axis=1,
    ).astype(np.float32) / float(H * W)

    chv = np.stack(
        [
            inputs["b_dce"], inputs["g_bn1"], inputs["be_bn1"],
            inputs["g_bn2"], inputs["be_bn2"], inputs["g_bns"],
            inputs["be_bns"], inputs["b_expand"],
        ],
        axis=1,
    ).astype(np.float32)

    shared = {
        "w1t": w1t, "w2t": w2t, "wsct": wsct, "wdce_t": wdce_t,
        "wst": wst, "wet": wet, "chc": np.ascontiguousarray(chc),
        "chv": np.ascontiguousarray(chv),
        "bsh": inputs["b_shrink"].astype(np.float32),
        "eye": np.eye(128, dtype=np.float16),
    }
    in_maps = []
    for c in range(NCORES):
        m = dict(shared)
        xc = inputs["x"][c * NB : (c + 1) * NB]
        xp = np.zeros((NB, C, PLEN), np.float32)
        xp[:, :, : (H + 2) * PW].reshape(NB, C, H + 2, PW)[
            :, :, 1 : H + 1, 1 : W + 1
        ] = xc
        m["xp"] = xp.astype(f16)
        m["dce"] = np.ascontiguousarray(
            inputs["dce_output"][c * NB : (c + 1) * NB]
        ).astype(np.float32)
        in_maps.append(m)
    return in_maps


def kernel(**inputs):
    global _NC
    if _NC is None:
        _NC = _build()
    in_maps = _prep_inputs(inputs)
    res = run_bass_kernel_spmd(_NC, in_maps, list(range(NCORES)))
    return np.concatenate([res.results[c]["y"] for c in range(NCORES)], axis=0)


if __name__ == "__main__":
    nc = _build()
    print("build ok")
